# revision 1
# baseline (speedup 1.0000x reference)
"""GCN (2x GCNConv + MLP head) on 8 TRN2 NeuronCores via Bass/Tile.

Distribution (graph-parallel, per the node-sharding scheme):
  - nodes sharded by id across 8 cores (12500 each); weights replicated.
  - Phase A (replicated): h1l rows = (dinv*x) @ W1 for ALL nodes -> DRAM.
  - Conv edge phase (sharded by dst): for each core's in-edges,
    dma_gather 256B message rows by src id, then per-128-edge block a
    DVE-built one-hot S_dst and a PE matmul accumulate aggT[64,128] per
    dst tile in PSUM (exact f32); epilogue h1T = dinv*aggT + b1.
  - AllGather of h1T shards (bf16) = the halo exchange.
  - Phase C (replicated): h2l rows = h1 @ W2 for ALL nodes -> DRAM.
  - Conv2 edge phase -> h2T (f32, SBUF resident).
  - MLP head in transposed space; output row [1, shard].

Host preprocessing is structure-only (derived from edge_index): degrees,
edge blocking by (dst-tile, src-window), int16 gather indices. All cores
share one program: block structure is padded to the max across cores.
"""

import numpy as np
import ml_dtypes

import concourse.bass as bass
import concourse.bacc as bacc
import concourse.tile as tile
import concourse.mybir as mybir
from concourse.bass_utils import run_bass_kernel_spmd

F32 = mybir.dt.float32
BF16 = mybir.dt.bfloat16
I16 = mybir.dt.int16

NCORES = 8
WIN = 25088          # gather window rows (multiple of 128, < int16 max)
GAP = 128            # zero rows appended per window (pad-edge target)
WSTRIDE = WIN + GAP
TILE = 128           # dst tile size
CB = 8               # max 128-edge blocks per dma_gather (1024-idx HW limit)


# ----------------------------------------------------------------------------
# host-side preprocessing (numpy only)
# ----------------------------------------------------------------------------

def wrap16x8(a):
    """[n] int16 -> [128, n//16]: idx i at [i%16, i//16], replicated x8."""
    w = np.ascontiguousarray(np.transpose(a.reshape(-1, 16), (1, 0)))
    return np.ascontiguousarray(np.tile(w, (8, 1)))


def preprocess(n, edge_index):
    """Uniform cross-core edge plan.

    Returns (dinv, plan, cores) where plan holds the shared structure
    (chunks/blocks/flags) and cores[c] holds per-core staged index arrays.
    """
    src = edge_index[0].astype(np.int64)
    dst = edge_index[1].astype(np.int64)

    deg = np.bincount(dst, minlength=n).astype(np.float64) + 1.0
    dinv = (1.0 / np.sqrt(deg)).astype(np.float32)

    shard = n // NCORES
    assert shard * NCORES == n and shard % 2 == 0
    ntiles = (shard + TILE - 1) // TILE
    dpad = ntiles * TILE
    nwin = (n + WIN - 1) // WIN

    loops = np.arange(n, dtype=np.int64)
    src = np.concatenate([src, loops])
    dst = np.concatenate([dst, loops])

    # per-core edge lists grouped by (dst tile, src window)
    per_core = []
    counts = np.zeros((NCORES, ntiles, nwin), np.int64)
    for c in range(NCORES):
        base = c * shard
        m = (dst >= base) & (dst < base + shard)
        s, d = src[m], dst[m] - base
        t_id = d // TILE
        w_id = s // WIN
        order = np.lexsort((w_id, t_id))
        s, d, t_id, w_id = s[order], d[order], t_id[order], w_id[order]
        np.add.at(counts[c], (t_id, w_id), 1)
        per_core.append((s, d, t_id, w_id))

    nb = (counts.max(axis=0) + TILE - 1) // TILE      # [ntiles, nwin] blocks

    # shared chunk/block structure, tile-major
    chunks = []   # (window, n_blocks, tile)
    blocks = []   # (tile, start, stop)
    for t in range(ntiles):
        tile_blocks = int(nb[t].sum())
        done = 0
        for w in range(nwin):
            g = int(nb[t, w])
            b0 = 0
            while b0 < g:
                k = min(CB, g - b0)
                chunks.append((w, k, t))
                for j in range(k):
                    bi = done + b0 + j
                    blocks.append((t, bi == 0, bi == tile_blocks - 1))
                b0 += k
            done += g
    goff, boff = [], []
    g0 = b0_ = 0
    for (w, k, t) in chunks:
        goff.append(g0); boff.append(b0_)
        g0 += k * TILE // 16
        b0_ += k

    # per-core staged arrays
    cores = []
    for c in range(NCORES):
        s, d, t_id, w_id = per_core[c]
        gidx = np.full((b0_ * TILE,), WIN, np.int16)    # default: pad row
        dstl = np.zeros((b0_ * TILE,), np.float32)
        # locate each core group inside the shared layout
        key = t_id * nwin + w_id
        cuts = np.flatnonzero(np.diff(key)) + 1
        starts = np.concatenate([[0], cuts]) if len(s) else np.array([], np.int64)
        ends = np.concatenate([cuts, [len(s)]]) if len(s) else np.array([], np.int64)
        # block offset of group (t, w) in the shared layout
        grp_boff = np.zeros((ntiles, nwin), np.int64)
        acc = 0
        for t in range(ntiles):
            for w in range(nwin):
                grp_boff[t, w] = acc
                acc += nb[t, w]
        for a, b in zip(starts, ends):
            t = int(t_id[a]); w = int(w_id[a])
            o = grp_boff[t, w] * TILE
            cnt = b - a
            gidx[o:o + cnt] = (s[a:b] - w * WIN).astype(np.int16)
            dstl[o:o + cnt] = (d[a:b] - t * TILE).astype(np.float32)
        cores.append(dict(
            gidx=wrap16x8(gidx),
            dstl=np.ascontiguousarray(dstl.reshape(b0_, TILE).T),
            base=c * shard,
        ))

    plan = dict(chunks=chunks, blocks=blocks, goff=goff, boff=boff,
                ntiles=ntiles, dpad=dpad, shard=shard, nwin=nwin,
                gcols=g0, bcols=b0_)
    return dinv, plan, cores


# ----------------------------------------------------------------------------
# device program
# ----------------------------------------------------------------------------

def emit_conv_edges(nc, pool, ipool, psum, plan, hbuf, gidx_d, dstl_d, iota_t,
                    dinvrep_t, bias_t, out_cb, out_dtype):
    """One conv's edge aggregation. out_cb(tile_idx, ap_or_tile)."""
    agg = {"t": None}
    bi = 0
    for ci, (w, k, t) in enumerate(plan["chunks"]):
        go = plan["goff"][ci]
        bo = plan["boff"][ci]
        nidx = k * TILE
        it = ipool.tile([128, CB * TILE // 16], I16, tag="gidx")
        nc.sync.dma_start(it[:, :nidx // 16], gidx_d[:, go:go + nidx // 16])
        dl = ipool.tile([128, CB], F32, tag="dstl")
        nc.sync.dma_start(dl[:, :k], dstl_d[:, bo:bo + k])
        g = pool.tile([128, CB, 64], F32, tag="g")
        nc.gpsimd.dma_gather(
            g[:, :k, :],
            hbuf[w * WSTRIDE:(w + 1) * WSTRIDE, :],
            it[:, :nidx // 16],
            num_idxs=nidx, num_idxs_reg=nidx, elem_size=64,
        )
        s_t = pool.tile([128, CB, TILE], F32, tag="s")
        nc.vector.tensor_tensor(
            s_t[:, :k, :],
            iota_t[:].unsqueeze(1).broadcast_to([128, k, TILE]),
            dl[:, :k].unsqueeze(2).broadcast_to([128, k, TILE]),
            op=mybir.AluOpType.is_equal,
        )
        for j in range(k):
            t_, start, stop = plan["blocks"][bi]; bi += 1
            if start:
                agg["t"] = psum.tile([64, TILE], F32, tag="agg", name=f"agg_{bi}")
            nc.tensor.matmul(agg["t"][:], lhsT=g[:, j, :], rhs=s_t[:, j, :],
                             start=start, stop=stop)
            if stop:
                ag = agg["t"]
                e1 = pool.tile([64, TILE], F32, tag="ep1")
                nc.vector.tensor_tensor(
                    e1[:], ag[:],
                    dinvrep_t[:, t_ * TILE:(t_ + 1) * TILE],
                    op=mybir.AluOpType.mult)
                e2 = pool.tile([64, TILE], out_dtype, tag="ep2")
                nc.vector.tensor_tensor(
                    e2[:], e1[:], bias_t[:].broadcast_to([64, TILE]),
                    op=mybir.AluOpType.add)
                out_cb(t_, e2)


def build_program(meta, plan):
    n = meta["n"]
    npad = meta["npad"]
    nwin = npad // WIN
    hrows = nwin * WSTRIDE
    dpad = plan["dpad"]
    shard = plan["shard"]
    ntiles = plan["ntiles"]
    gcols = max(plan["gcols"], 16)
    bcols = max(plan["bcols"], 1)

    nc = bacc.Bacc("TRN2", target_bir_lowering=False, debug=False,
                   num_devices=NCORES)

    xt = nc.dram_tensor("xt", [128, npad], BF16, kind="ExternalInput")
    h1buf = nc.dram_tensor("h1buf", [hrows, 64], F32, kind="ExternalInput")
    h2buf = nc.dram_tensor("h2buf", [hrows, 64], F32, kind="ExternalInput")
    gidx_d = nc.dram_tensor("gidx", [128, gcols], I16, kind="ExternalInput")
    dstl_d = nc.dram_tensor("dstl", [128, bcols], F32, kind="ExternalInput")
    w1_d = nc.dram_tensor("w1", [128, 64], BF16, kind="ExternalInput")
    w2_d = nc.dram_tensor("w2", [64, 64], BF16, kind="ExternalInput")
    lw1_d = nc.dram_tensor("lw1", [64, 64], F32, kind="ExternalInput")
    lw2_d = nc.dram_tensor("lw2", [64, 32], F32, kind="ExternalInput")
    lw3_d = nc.dram_tensor("lw3", [32, 1], F32, kind="ExternalInput")
    b1_d = nc.dram_tensor("b1", [64, 1], F32, kind="ExternalInput")
    b2_d = nc.dram_tensor("b2", [64, 1], F32, kind="ExternalInput")
    lb1_d = nc.dram_tensor("lb1", [64, 1], F32, kind="ExternalInput")
    lb2_d = nc.dram_tensor("lb2", [32, 1], F32, kind="ExternalInput")
    lb3_d = nc.dram_tensor("lb3", [1, 1], F32, kind="ExternalInput")
    iota_d = nc.dram_tensor("iota", [128, TILE], F32, kind="ExternalInput")
    dinvrep_d = nc.dram_tensor("dinvrep", [64, dpad], F32, kind="ExternalInput")
    dinvc_d = nc.dram_tensor("dinvc", [128, NCORES * ntiles], F32,
                             kind="ExternalInput")
    out_d = nc.dram_tensor("out", [1, dpad], F32, kind="ExternalOutput")

    with tile.TileContext(nc) as tc:
        with (
            tc.tile_pool(name="const", bufs=1) as cpool,
            tc.tile_pool(name="work", bufs=6) as pool,
            tc.tile_pool(name="idx", bufs=6) as ipool,
            tc.tile_pool(name="xtp", bufs=4) as xtpool,
            tc.tile_pool(name="psag", bufs=2, space="PSUM") as psag,
            tc.tile_pool(name="psmm", bufs=4, space="PSUM") as psmm,
            tc.tile_pool(name="dram", bufs=1, space="DRAM") as dram,
        ):
            def load_const(dram_t, shape, dtype, tag):
                t = cpool.tile(shape, dtype, tag=tag)
                nc.sync.dma_start(t[:], dram_t[:])
                return t

            w1_t = load_const(w1_d, [128, 64], BF16, "w1")
            w2_t = load_const(w2_d, [64, 64], BF16, "w2")
            lw1_t = load_const(lw1_d, [64, 64], F32, "lw1")
            lw2_t = load_const(lw2_d, [64, 32], F32, "lw2")
            lw3_t = load_const(lw3_d, [32, 1], F32, "lw3")
            b1_t = load_const(b1_d, [64, 1], F32, "b1")
            b2_t = load_const(b2_d, [64, 1], F32, "b2")
            lb1_t = load_const(lb1_d, [64, 1], F32, "lb1")
            lb2_t = load_const(lb2_d, [32, 1], F32, "lb2")
            lb3_t = load_const(lb3_d, [1, 1], F32, "lb3")
            iota_t = load_const(iota_d, [128, TILE], F32, "iota")
            dinvrep_t = load_const(dinvrep_d, [64, dpad], F32, "dinvrep")
            dinvc_t = load_const(dinvc_d, [128, NCORES * ntiles], F32, "dinvc")

            # --- phase A ---
            for t in range(npad // TILE):
                st = xtpool.tile([128, TILE], BF16, tag="xt")
                nc.sync.dma_start(st[:], xt[:, t * TILE:(t + 1) * TILE])
                ps = psmm.tile([TILE, 64], F32, tag="mm")
                nc.tensor.matmul(ps[:], lhsT=st[:], rhs=w1_t[:],
                                 start=True, stop=True)
                sb = pool.tile([TILE, 64], F32, tag="arow")
                nc.vector.tensor_copy(sb[:], ps[:])
                w = (t * TILE) // WIN
                r = w * WSTRIDE + (t * TILE) % WIN
                nc.sync.dma_start(h1buf[r:r + TILE, :], sb[:])

            # --- conv1 edges -> h1T bf16 bounce ---
            h1t_bounce = dram.tile([64, dpad], BF16)
            ag_out = dram.tile([NCORES * 64, dpad], BF16, addr_space="Shared")

            def conv1_out(t_, e2):
                nc.sync.dma_start(h1t_bounce[:, t_ * TILE:(t_ + 1) * TILE], e2[:])

            emit_conv_edges(nc, pool, ipool, psag, plan, h1buf, gidx_d, dstl_d,
                            iota_t, dinvrep_t, b1_t, conv1_out, BF16)

            if dpad > shard:
                zt = pool.tile([64, dpad - shard], BF16, tag="zt")
                nc.vector.memset(zt[:], 0.0)
                nc.sync.dma_start(h1t_bounce[:, shard:], zt[:])

            nc.gpsimd.collective_compute(
                "AllGather", mybir.AluOpType.bypass,
                ins=[h1t_bounce[:].opt()],
                outs=[ag_out[:].opt()],
                replica_groups=[list(range(NCORES))],
            )

            # --- phase C: h2l rows for all nodes ---
            for c in range(NCORES):
                for t in range(ntiles):
                    n0 = c * shard + t * TILE
                    cnt = min(TILE, shard - t * TILE)
                    st = xtpool.tile([64, TILE], BF16, tag="ct")
                    nc.sync.dma_start(
                        st[:, :cnt],
                        ag_out[c * 64:(c + 1) * 64, t * TILE:t * TILE + cnt])
                    ps = psmm.tile([TILE, 64], F32, tag="mm")
                    nc.tensor.matmul(ps[:cnt, :], lhsT=st[:, :cnt], rhs=w2_t[:],
                                     start=True, stop=True)
                    sb = pool.tile([TILE, 64], F32, tag="crow")
                    nc.vector.tensor_tensor(
                        sb[:cnt, :], ps[:cnt, :],
                        dinvc_t[:cnt, c * ntiles + t:c * ntiles + t + 1]
                        .broadcast_to([cnt, 64]),
                        op=mybir.AluOpType.mult)
                    off = 0
                    while off < cnt:
                        nn = n0 + off
                        w = nn // WIN
                        take = min(cnt - off, (w + 1) * WIN - nn)
                        r = w * WSTRIDE + (nn % WIN)
                        nc.sync.dma_start(h2buf[r:r + take, :],
                                          sb[off:off + take, :])
                        off += take

            # --- conv2 edges -> h2T f32 in SBUF ---
            h2t_sb = cpool.tile([64, dpad], F32, tag="h2t")

            def conv2_out(t_, e2):
                nc.vector.tensor_copy(h2t_sb[:, t_ * TILE:(t_ + 1) * TILE],
                                      e2[:])

            emit_conv_edges(nc, pool, ipool, psag, plan, h2buf, gidx_d, dstl_d,
                            iota_t, dinvrep_t, b2_t, conv2_out, F32)

            # --- MLP head (transposed space) ---
            EC = 512
            for o in range(0, dpad, EC):
                w_ = min(EC, dpad - o)
                p1 = psmm.tile([64, EC], F32, tag="mm")
                nc.tensor.matmul(p1[:, :w_], lhsT=lw1_t[:],
                                 rhs=h2t_sb[:, o:o + w_], start=True, stop=True)
                z1 = pool.tile([64, EC], F32, tag="z1")
                nc.scalar.activation(z1[:, :w_], p1[:, :w_],
                                     mybir.ActivationFunctionType.Relu,
                                     bias=lb1_t[:])
                p2 = psmm.tile([32, EC], F32, tag="mm")
                nc.tensor.matmul(p2[:, :w_], lhsT=lw2_t[:], rhs=z1[:, :w_],
                                 start=True, stop=True)
                z2 = pool.tile([32, EC], F32, tag="z2")
                nc.scalar.activation(z2[:, :w_], p2[:, :w_],
                                     mybir.ActivationFunctionType.Relu,
                                     bias=lb2_t[:])
                p3 = psmm.tile([1, EC], F32, tag="mm")
                nc.tensor.matmul(p3[:, :w_], lhsT=lw3_t[:], rhs=z2[:, :w_],
                                 start=True, stop=True)
                z3 = pool.tile([1, EC], F32, tag="z3")
                nc.vector.tensor_tensor(z3[:, :w_], p3[:, :w_],
                                        lb3_t[:].broadcast_to([1, w_]),
                                        op=mybir.AluOpType.add)
                nc.sync.dma_start(out_d[:, o:o + w_], z3[:, :w_])

    nc.compile()
    return nc


# ----------------------------------------------------------------------------
# entry point
# ----------------------------------------------------------------------------

def kernel(x, edge_index, W1, b1, W2, b2, lw1, lb1, lw2, lb2, lw3, lb3,
           _want_trace=False):
    x = np.asarray(x, np.float32)
    edge_index = np.asarray(edge_index)
    n = x.shape[0]
    npad = ((n + WIN - 1) // WIN) * WIN
    nwin = npad // WIN
    hrows = nwin * WSTRIDE

    dinv, plan, cores = preprocess(n, edge_index)
    shard, dpad, ntiles = plan["shard"], plan["dpad"], plan["ntiles"]

    xt = np.zeros((128, npad), ml_dtypes.bfloat16)
    xt[:, :n] = (x * dinv[:, None]).T.astype(ml_dtypes.bfloat16)
    hz = np.zeros((hrows, 64), np.float32)
    iota = np.tile(np.arange(TILE, dtype=np.float32), (128, 1))

    dinvc = np.zeros((128, NCORES * ntiles), np.float32)
    for cc in range(NCORES):
        for t in range(ntiles):
            n0 = cc * shard + t * TILE
            cnt = min(TILE, (cc + 1) * shard - n0)
            dinvc[:cnt, cc * ntiles + t] = dinv[n0:n0 + cnt]

    in_maps = []
    for c in range(NCORES):
        dinvrep = np.zeros((64, dpad), np.float32)
        dinvrep[:, :shard] = dinv[c * shard:(c + 1) * shard][None, :]
        in_maps.append({
            "xt": xt, "h1buf": hz, "h2buf": hz,
            "gidx": cores[c]["gidx"], "dstl": cores[c]["dstl"],
            "w1": np.asarray(W1, np.float32).astype(ml_dtypes.bfloat16),
            "w2": np.asarray(W2, np.float32).astype(ml_dtypes.bfloat16),
            "lw1": np.ascontiguousarray(np.asarray(lw1, np.float32)),
            "lw2": np.ascontiguousarray(np.asarray(lw2, np.float32)),
            "lw3": np.ascontiguousarray(np.asarray(lw3, np.float32)),
            "b1": np.asarray(b1, np.float32).reshape(-1, 1),
            "b2": np.asarray(b2, np.float32).reshape(-1, 1),
            "lb1": np.asarray(lb1, np.float32).reshape(-1, 1),
            "lb2": np.asarray(lb2, np.float32).reshape(-1, 1),
            "lb3": np.asarray(lb3, np.float32).reshape(-1, 1),
            "iota": iota, "dinvrep": dinvrep, "dinvc": dinvc,
        })

    meta = {"n": n, "npad": npad}
    nc = build_program(meta, plan)

    res = run_bass_kernel_spmd(nc, in_maps, core_ids=list(range(NCORES)),
                               trace=_want_trace)
    out = np.empty((n, 1), np.float32)
    for c in range(NCORES):
        out[c * shard:(c + 1) * shard, 0] = res.results[c]["out"][0, :shard]
    kernel._last_exec_ns = res.exec_time_ns
    return out



# revision 2
# speedup vs baseline: 1.6358x; 1.6358x over previous
"""GCN (2x GCNConv + MLP head) on 8 TRN2 NeuronCores via Bass/Tile.

Distribution (graph-parallel, per the node-sharding scheme):
  - nodes sharded by id across 8 cores (12500 each); weights replicated.
  - Phase A (replicated): h1l rows = (dinv*x) @ W1 for ALL nodes -> DRAM.
  - Conv edge phase (sharded by dst): for each core's in-edges,
    dma_gather 256B message rows by src id (4 SWDGE queues round-robin),
    scalar-engine copy to bf16, then per-128-edge block a DVE-built
    bf16 one-hot S_dst and a bf16 PE matmul accumulate aggT[64,128] per
    dst tile in PSUM; epilogue h1T = dinv*aggT + b1.
  - AllGather of h1T shards (bf16) = the halo exchange.
  - Phase C (replicated): h2l rows = h1 @ W2 for ALL nodes -> DRAM.
  - Conv2 edge phase -> h2T (f32, SBUF resident).
  - MLP head in transposed space; output row [1, shard].

Host preprocessing is structure-only (derived from edge_index): degrees,
edge blocking by (dst-tile, src-window), int16 gather indices. All cores
share one program: block structure is padded to the max across cores.
Pad slots gather window row 0 and carry dstl=-1 so the one-hot zeroes
their contribution.
"""

import numpy as np
import ml_dtypes

import concourse.bass as bass
import concourse.bacc as bacc
import concourse.tile as tile
import concourse.mybir as mybir
from concourse.bass_utils import run_bass_kernel_spmd

F32 = mybir.dt.float32
BF16 = mybir.dt.bfloat16
I16 = mybir.dt.int16

NCORES = 8
WIN = 25088          # gather window rows (multiple of 128, < int16 max)
TILE = 128           # dst tile size
CB = 8               # max 128-edge blocks per dma_gather (1024-idx HW limit)
NQ = 4               # SWDGE queues (ucode max)


# ----------------------------------------------------------------------------
# host-side preprocessing (numpy only)
# ----------------------------------------------------------------------------

def wrap16x8(a):
    """[n] int16 -> [128, n//16]: idx i at [i%16, i//16], replicated x8."""
    w = np.ascontiguousarray(np.transpose(a.reshape(-1, 16), (1, 0)))
    return np.ascontiguousarray(np.tile(w, (8, 1)))


def preprocess(n, edge_index):
    """Uniform cross-core edge plan.

    Returns (dinv, plan, cores) where plan holds the shared structure
    (chunks/blocks/flags) and cores[c] holds per-core staged index arrays.
    """
    src = edge_index[0].astype(np.int64)
    dst = edge_index[1].astype(np.int64)

    deg = np.bincount(dst, minlength=n).astype(np.float64) + 1.0
    dinv = (1.0 / np.sqrt(deg)).astype(np.float32)

    shard = n // NCORES
    assert shard * NCORES == n and shard % 2 == 0
    ntiles = (shard + TILE - 1) // TILE
    dpad = ntiles * TILE
    nwin = (n + WIN - 1) // WIN

    loops = np.arange(n, dtype=np.int64)
    src = np.concatenate([src, loops])
    dst = np.concatenate([dst, loops])

    # per-core edge lists grouped by (dst tile, src window)
    per_core = []
    counts = np.zeros((NCORES, ntiles, nwin), np.int64)
    for c in range(NCORES):
        base = c * shard
        m = (dst >= base) & (dst < base + shard)
        s, d = src[m], dst[m] - base
        t_id = d // TILE
        w_id = s // WIN
        order = np.lexsort((w_id, t_id))
        s, d, t_id, w_id = s[order], d[order], t_id[order], w_id[order]
        np.add.at(counts[c], (t_id, w_id), 1)
        per_core.append((s, d, t_id, w_id))

    nb = (counts.max(axis=0) + TILE - 1) // TILE      # [ntiles, nwin] blocks

    # shared chunk/block structure, tile-major
    chunks = []   # (window, n_blocks, tile)
    blocks = []   # (tile, start, stop)
    for t in range(ntiles):
        tile_blocks = int(nb[t].sum())
        done = 0
        for w in range(nwin):
            g = int(nb[t, w])
            b0 = 0
            while b0 < g:
                k = min(CB, g - b0)
                chunks.append((w, k, t))
                for j in range(k):
                    bi = done + b0 + j
                    blocks.append((t, bi == 0, bi == tile_blocks - 1))
                b0 += k
            done += g
    goff, boff = [], []
    g0 = b0_ = 0
    for (w, k, t) in chunks:
        goff.append(g0); boff.append(b0_)
        g0 += k * TILE // 16
        b0_ += k

    # per-core staged arrays
    cores = []
    for c in range(NCORES):
        s, d, t_id, w_id = per_core[c]
        gidx = np.zeros((b0_ * TILE,), np.int16)       # pad: window row 0
        dstl = np.full((b0_ * TILE,), -1.0, np.float32)  # pad: matches no dst
        # locate each core group inside the shared layout
        key = t_id * nwin + w_id
        cuts = np.flatnonzero(np.diff(key)) + 1
        starts = np.concatenate([[0], cuts]) if len(s) else np.array([], np.int64)
        ends = np.concatenate([cuts, [len(s)]]) if len(s) else np.array([], np.int64)
        # block offset of group (t, w) in the shared layout
        grp_boff = np.zeros((ntiles, nwin), np.int64)
        acc = 0
        for t in range(ntiles):
            for w in range(nwin):
                grp_boff[t, w] = acc
                acc += nb[t, w]
        for a, b in zip(starts, ends):
            t = int(t_id[a]); w = int(w_id[a])
            o = grp_boff[t, w] * TILE
            cnt = b - a
            gidx[o:o + cnt] = (s[a:b] - w * WIN).astype(np.int16)
            dstl[o:o + cnt] = (d[a:b] - t * TILE).astype(np.float32)
        cores.append(dict(
            gidx=wrap16x8(gidx),
            dstl=np.ascontiguousarray(dstl.reshape(b0_, TILE).T),
            base=c * shard,
        ))

    plan = dict(chunks=chunks, blocks=blocks, goff=goff, boff=boff,
                ntiles=ntiles, dpad=dpad, shard=shard, nwin=nwin,
                gcols=g0, bcols=b0_)
    return dinv, plan, cores


# ----------------------------------------------------------------------------
# device program
# ----------------------------------------------------------------------------

def emit_conv_edges(nc, pool, ipool, psum, plan, hbuf, gidx_d, dstl_d, iota_t,
                    dinvrep_t, bias_t, out_cb, out_dtype):
    """One conv's edge aggregation. out_cb(tile_idx, ap_or_tile)."""
    agg = {"t": None}
    bi = 0
    for ci, (w, k, t) in enumerate(plan["chunks"]):
        go = plan["goff"][ci]
        bo = plan["boff"][ci]
        nidx = k * TILE
        it = ipool.tile([128, CB * TILE // 16], I16, tag="gidx")
        nc.sync.dma_start(it[:, :nidx // 16], gidx_d[:, go:go + nidx // 16])
        dl = ipool.tile([128, CB], F32, tag="dstl")
        nc.sync.dma_start(dl[:, :k], dstl_d[:, bo:bo + k])
        g = pool.tile([128, CB, 64], F32, tag="g")
        nc.gpsimd.dma_gather(
            g[:, :k, :],
            hbuf[w * WIN:(w + 1) * WIN, :],
            it[:, :nidx // 16],
            num_idxs=nidx, num_idxs_reg=nidx, elem_size=64,
            queue_num=ci % NQ,
        )
        g2 = pool.tile([128, CB, 64], BF16, tag="g2")
        nc.scalar.activation(g2[:, :k, :], g[:, :k, :],
                             mybir.ActivationFunctionType.Copy)
        s_t = pool.tile([128, CB, TILE], BF16, tag="s")
        nc.vector.tensor_tensor(
            s_t[:, :k, :],
            iota_t[:].unsqueeze(1).broadcast_to([128, k, TILE]),
            dl[:, :k].unsqueeze(2).broadcast_to([128, k, TILE]),
            op=mybir.AluOpType.is_equal,
        )
        for j in range(k):
            t_, start, stop = plan["blocks"][bi]; bi += 1
            if start:
                agg["t"] = psum.tile([64, TILE], F32, tag="agg", name=f"agg_{bi}")
            nc.tensor.matmul(agg["t"][:], lhsT=g2[:, j, :], rhs=s_t[:, j, :],
                             start=start, stop=stop)
            if stop:
                ag = agg["t"]
                e1 = pool.tile([64, TILE], F32, tag="ep1")
                nc.vector.tensor_tensor(
                    e1[:], ag[:],
                    dinvrep_t[:, t_ * TILE:(t_ + 1) * TILE],
                    op=mybir.AluOpType.mult)
                e2 = pool.tile([64, TILE], out_dtype, tag="ep2")
                nc.vector.tensor_tensor(
                    e2[:], e1[:], bias_t[:].broadcast_to([64, TILE]),
                    op=mybir.AluOpType.add)
                out_cb(t_, e2)


def build_program(meta, plan):
    n = meta["n"]
    npad = meta["npad"]
    dpad = plan["dpad"]
    shard = plan["shard"]
    ntiles = plan["ntiles"]
    gcols = max(plan["gcols"], 16)
    bcols = max(plan["bcols"], 1)

    nc = bacc.Bacc("TRN2", target_bir_lowering=False, debug=False,
                   num_devices=NCORES, num_swdge_queues=NQ)

    xt = nc.dram_tensor("xt", [128, npad], BF16, kind="ExternalInput")
    h1buf = nc.dram_tensor("h1buf", [npad, 64], F32, kind="ExternalInput")
    h2buf = nc.dram_tensor("h2buf", [npad, 64], F32, kind="ExternalInput")
    gidx_d = nc.dram_tensor("gidx", [128, gcols], I16, kind="ExternalInput")
    dstl_d = nc.dram_tensor("dstl", [128, bcols], F32, kind="ExternalInput")
    w1_d = nc.dram_tensor("w1", [128, 64], BF16, kind="ExternalInput")
    w2_d = nc.dram_tensor("w2", [64, 64], BF16, kind="ExternalInput")
    lw1_d = nc.dram_tensor("lw1", [64, 64], F32, kind="ExternalInput")
    lw2_d = nc.dram_tensor("lw2", [64, 32], F32, kind="ExternalInput")
    lw3_d = nc.dram_tensor("lw3", [32, 1], F32, kind="ExternalInput")
    b1_d = nc.dram_tensor("b1", [64, 1], F32, kind="ExternalInput")
    b2_d = nc.dram_tensor("b2", [64, 1], F32, kind="ExternalInput")
    lb1_d = nc.dram_tensor("lb1", [64, 1], F32, kind="ExternalInput")
    lb2_d = nc.dram_tensor("lb2", [32, 1], F32, kind="ExternalInput")
    lb3_d = nc.dram_tensor("lb3", [1, 1], F32, kind="ExternalInput")
    iota_d = nc.dram_tensor("iota", [128, TILE], F32, kind="ExternalInput")
    dinvrep_d = nc.dram_tensor("dinvrep", [64, dpad], F32, kind="ExternalInput")
    dinvc_d = nc.dram_tensor("dinvc", [128, NCORES * ntiles], F32,
                             kind="ExternalInput")
    out_d = nc.dram_tensor("out", [1, dpad], F32, kind="ExternalOutput")

    with tile.TileContext(nc) as tc:
        with (
            tc.tile_pool(name="const", bufs=1) as cpool,
            tc.tile_pool(name="work", bufs=8) as pool,
            tc.tile_pool(name="idx", bufs=8) as ipool,
            tc.tile_pool(name="xtp", bufs=4) as xtpool,
            tc.tile_pool(name="psag", bufs=2, space="PSUM") as psag,
            tc.tile_pool(name="psmm", bufs=4, space="PSUM") as psmm,
            tc.tile_pool(name="dram", bufs=1, space="DRAM") as dram,
        ):
            def load_const(dram_t, shape, dtype, tag):
                t = cpool.tile(shape, dtype, tag=tag)
                nc.sync.dma_start(t[:], dram_t[:])
                return t

            w1_t = load_const(w1_d, [128, 64], BF16, "w1")
            w2_t = load_const(w2_d, [64, 64], BF16, "w2")
            lw1_t = load_const(lw1_d, [64, 64], F32, "lw1")
            lw2_t = load_const(lw2_d, [64, 32], F32, "lw2")
            lw3_t = load_const(lw3_d, [32, 1], F32, "lw3")
            b1_t = load_const(b1_d, [64, 1], F32, "b1")
            b2_t = load_const(b2_d, [64, 1], F32, "b2")
            lb1_t = load_const(lb1_d, [64, 1], F32, "lb1")
            lb2_t = load_const(lb2_d, [32, 1], F32, "lb2")
            lb3_t = load_const(lb3_d, [1, 1], F32, "lb3")
            iota_t = load_const(iota_d, [128, TILE], F32, "iota")
            dinvrep_t = load_const(dinvrep_d, [64, dpad], F32, "dinvrep")
            dinvc_t = load_const(dinvc_d, [128, NCORES * ntiles], F32, "dinvc")

            # --- phase A ---
            for t in range(npad // TILE):
                st = xtpool.tile([128, TILE], BF16, tag="xt")
                nc.sync.dma_start(st[:], xt[:, t * TILE:(t + 1) * TILE])
                ps = psmm.tile([TILE, 64], F32, tag="mm")
                nc.tensor.matmul(ps[:], lhsT=st[:], rhs=w1_t[:],
                                 start=True, stop=True)
                sb = pool.tile([TILE, 64], F32, tag="arow")
                nc.vector.tensor_copy(sb[:], ps[:])
                nc.sync.dma_start(h1buf[t * TILE:(t + 1) * TILE, :], sb[:])

            # --- conv1 edges -> h1T bf16 bounce ---
            h1t_bounce = dram.tile([64, dpad], BF16)
            ag_out = dram.tile([NCORES * 64, dpad], BF16, addr_space="Shared")

            def conv1_out(t_, e2):
                nc.sync.dma_start(h1t_bounce[:, t_ * TILE:(t_ + 1) * TILE], e2[:])

            emit_conv_edges(nc, pool, ipool, psag, plan, h1buf, gidx_d, dstl_d,
                            iota_t, dinvrep_t, b1_t, conv1_out, BF16)

            if dpad > shard:
                zt = pool.tile([64, dpad - shard], BF16, tag="zt")
                nc.vector.memset(zt[:], 0.0)
                nc.sync.dma_start(h1t_bounce[:, shard:], zt[:])

            nc.gpsimd.collective_compute(
                "AllGather", mybir.AluOpType.bypass,
                ins=[h1t_bounce[:].opt()],
                outs=[ag_out[:].opt()],
                replica_groups=[list(range(NCORES))],
            )

            # --- phase C: h2l rows for all nodes ---
            for c in range(NCORES):
                for t in range(ntiles):
                    n0 = c * shard + t * TILE
                    cnt = min(TILE, shard - t * TILE)
                    st = xtpool.tile([64, TILE], BF16, tag="ct")
                    nc.sync.dma_start(
                        st[:, :cnt],
                        ag_out[c * 64:(c + 1) * 64, t * TILE:t * TILE + cnt])
                    ps = psmm.tile([TILE, 64], F32, tag="mm")
                    nc.tensor.matmul(ps[:cnt, :], lhsT=st[:, :cnt], rhs=w2_t[:],
                                     start=True, stop=True)
                    sb = pool.tile([TILE, 64], F32, tag="crow")
                    nc.vector.tensor_tensor(
                        sb[:cnt, :], ps[:cnt, :],
                        dinvc_t[:cnt, c * ntiles + t:c * ntiles + t + 1]
                        .broadcast_to([cnt, 64]),
                        op=mybir.AluOpType.mult)
                    nc.sync.dma_start(h2buf[n0:n0 + cnt, :], sb[:cnt, :])

            # --- conv2 edges -> h2T f32 in SBUF ---
            h2t_sb = cpool.tile([64, dpad], F32, tag="h2t")

            def conv2_out(t_, e2):
                nc.vector.tensor_copy(h2t_sb[:, t_ * TILE:(t_ + 1) * TILE],
                                      e2[:])

            emit_conv_edges(nc, pool, ipool, psag, plan, h2buf, gidx_d, dstl_d,
                            iota_t, dinvrep_t, b2_t, conv2_out, F32)

            # --- MLP head (transposed space) ---
            EC = 512
            for o in range(0, dpad, EC):
                w_ = min(EC, dpad - o)
                p1 = psmm.tile([64, EC], F32, tag="mm")
                nc.tensor.matmul(p1[:, :w_], lhsT=lw1_t[:],
                                 rhs=h2t_sb[:, o:o + w_], start=True, stop=True)
                z1 = pool.tile([64, EC], F32, tag="z1")
                nc.scalar.activation(z1[:, :w_], p1[:, :w_],
                                     mybir.ActivationFunctionType.Relu,
                                     bias=lb1_t[:])
                p2 = psmm.tile([32, EC], F32, tag="mm")
                nc.tensor.matmul(p2[:, :w_], lhsT=lw2_t[:], rhs=z1[:, :w_],
                                 start=True, stop=True)
                z2 = pool.tile([32, EC], F32, tag="z2")
                nc.scalar.activation(z2[:, :w_], p2[:, :w_],
                                     mybir.ActivationFunctionType.Relu,
                                     bias=lb2_t[:])
                p3 = psmm.tile([1, EC], F32, tag="mm")
                nc.tensor.matmul(p3[:, :w_], lhsT=lw3_t[:], rhs=z2[:, :w_],
                                 start=True, stop=True)
                z3 = pool.tile([1, EC], F32, tag="z3")
                nc.vector.tensor_tensor(z3[:, :w_], p3[:, :w_],
                                        lb3_t[:].broadcast_to([1, w_]),
                                        op=mybir.AluOpType.add)
                nc.sync.dma_start(out_d[:, o:o + w_], z3[:, :w_])

    nc.compile()
    return nc


# ----------------------------------------------------------------------------
# entry point
# ----------------------------------------------------------------------------

def kernel(x, edge_index, W1, b1, W2, b2, lw1, lb1, lw2, lb2, lw3, lb3,
           _want_trace=False):
    x = np.asarray(x, np.float32)
    edge_index = np.asarray(edge_index)
    n = x.shape[0]
    npad = ((n + WIN - 1) // WIN) * WIN

    dinv, plan, cores = preprocess(n, edge_index)
    shard, dpad, ntiles = plan["shard"], plan["dpad"], plan["ntiles"]

    xt = np.zeros((128, npad), ml_dtypes.bfloat16)
    xt[:, :n] = (x * dinv[:, None]).T.astype(ml_dtypes.bfloat16)
    hz = np.zeros((npad, 64), np.float32)
    iota = np.tile(np.arange(TILE, dtype=np.float32), (128, 1))

    dinvc = np.zeros((128, NCORES * ntiles), np.float32)
    for cc in range(NCORES):
        for t in range(ntiles):
            n0 = cc * shard + t * TILE
            cnt = min(TILE, (cc + 1) * shard - n0)
            dinvc[:cnt, cc * ntiles + t] = dinv[n0:n0 + cnt]

    in_maps = []
    for c in range(NCORES):
        dinvrep = np.zeros((64, dpad), np.float32)
        dinvrep[:, :shard] = dinv[c * shard:(c + 1) * shard][None, :]
        in_maps.append({
            "xt": xt, "h1buf": hz, "h2buf": hz,
            "gidx": cores[c]["gidx"], "dstl": cores[c]["dstl"],
            "w1": np.asarray(W1, np.float32).astype(ml_dtypes.bfloat16),
            "w2": np.asarray(W2, np.float32).astype(ml_dtypes.bfloat16),
            "lw1": np.ascontiguousarray(np.asarray(lw1, np.float32)),
            "lw2": np.ascontiguousarray(np.asarray(lw2, np.float32)),
            "lw3": np.ascontiguousarray(np.asarray(lw3, np.float32)),
            "b1": np.asarray(b1, np.float32).reshape(-1, 1),
            "b2": np.asarray(b2, np.float32).reshape(-1, 1),
            "lb1": np.asarray(lb1, np.float32).reshape(-1, 1),
            "lb2": np.asarray(lb2, np.float32).reshape(-1, 1),
            "lb3": np.asarray(lb3, np.float32).reshape(-1, 1),
            "iota": iota, "dinvrep": dinvrep, "dinvc": dinvc,
        })

    meta = {"n": n, "npad": npad}
    nc = build_program(meta, plan)

    res = run_bass_kernel_spmd(nc, in_maps, core_ids=list(range(NCORES)),
                               trace=_want_trace)
    out = np.empty((n, 1), np.float32)
    for c in range(NCORES):
        out[c * shard:(c + 1) * shard, 0] = res.results[c]["out"][0, :shard]
    kernel._last_exec_ns = res.exec_time_ns
    return out


# revision 6
# speedup vs baseline: 2.0868x; 1.2757x over previous
"""GCN (2x GCNConv + MLP head) on 8 TRN2 NeuronCores via Bass/Tile.

Distribution (graph-parallel, per the node-sharding scheme):
  - nodes sharded by id across 8 cores (12500 each); weights replicated.
  - Phase A (replicated): h1l rows = (dinv*x) @ W1 for ALL nodes -> DRAM.
  - Conv edge phase (sharded by dst): for each core's in-edges,
    dma_gather 256B message rows by src id (4 SWDGE queues round-robin),
    scalar-engine copy to bf16, then per-128-edge block a DVE-built
    bf16 one-hot S_dst and a bf16 PE matmul accumulate aggT[64,128] per
    dst tile in PSUM; epilogue h1T = dinv*aggT + b1.
  - AllGather of h1T shards (bf16) = the halo exchange.
  - Phase C (replicated): h2l rows = h1 @ W2 for ALL nodes -> DRAM.
  - Conv2 edge phase -> h2T (f32, SBUF resident).
  - MLP head in transposed space; output row [1, shard].

Host preprocessing is structure-only (derived from edge_index): degrees,
edge blocking by (dst-tile, src-window), int16 gather indices. All cores
share one program: block structure is padded to the max across cores.
Pad slots gather window row 0 and carry dstl=-1 so the one-hot zeroes
their contribution.
"""

import numpy as np
import ml_dtypes

import concourse.bass as bass
import concourse.bacc as bacc
import concourse.tile as tile
import concourse.mybir as mybir
from concourse.bass_utils import run_bass_kernel_spmd

F32 = mybir.dt.float32
BF16 = mybir.dt.bfloat16
I16 = mybir.dt.int16

NCORES = 8
WIN = 25088          # gather window rows (multiple of 128, < int16 max)
TILE = 128           # dst tile size
CB = 8               # max 128-edge blocks per dma_gather (1024-idx HW limit)
NQ = 4               # SWDGE queues (ucode max)
SG = 8               # chunks per coalesced index-load supergroup


# ----------------------------------------------------------------------------
# host-side preprocessing (numpy only)
# ----------------------------------------------------------------------------

def wrap16x8(a):
    """[n] int16 -> [128, n//16]: idx i at [i%16, i//16], replicated x8."""
    w = np.ascontiguousarray(np.transpose(a.reshape(-1, 16), (1, 0)))
    return np.ascontiguousarray(np.tile(w, (8, 1)))


def preprocess(n, edge_index):
    """Uniform cross-core edge plan.

    Returns (dinv, plan, cores) where plan holds the shared structure
    (chunks/blocks/flags) and cores[c] holds per-core staged index arrays.
    """
    src = edge_index[0].astype(np.int64)
    dst = edge_index[1].astype(np.int64)

    deg = np.bincount(dst, minlength=n).astype(np.float64) + 1.0
    dinv = (1.0 / np.sqrt(deg)).astype(np.float32)

    shard = n // NCORES
    assert shard * NCORES == n and shard % 2 == 0
    ntiles = (shard + TILE - 1) // TILE
    dpad = ntiles * TILE
    nwin = (n + WIN - 1) // WIN

    loops = np.arange(n, dtype=np.int64)
    src = np.concatenate([src, loops])
    dst = np.concatenate([dst, loops])

    # per-core edge lists grouped by (dst tile, src window)
    per_core = []
    counts = np.zeros((NCORES, ntiles, nwin), np.int64)
    for c in range(NCORES):
        base = c * shard
        m = (dst >= base) & (dst < base + shard)
        s, d = src[m], dst[m] - base
        t_id = d // TILE
        w_id = s // WIN
        order = np.lexsort((w_id, t_id))
        s, d, t_id, w_id = s[order], d[order], t_id[order], w_id[order]
        np.add.at(counts[c], (t_id, w_id), 1)
        per_core.append((s, d, t_id, w_id))

    nb = (counts.max(axis=0) + TILE - 1) // TILE      # [ntiles, nwin] blocks

    # shared chunk/block structure, tile-major
    chunks = []   # (window, n_blocks, tile)
    blocks = []   # (tile, start, stop)
    for t in range(ntiles):
        tile_blocks = int(nb[t].sum())
        done = 0
        for w in range(nwin):
            g = int(nb[t, w])
            b0 = 0
            while b0 < g:
                k = min(CB, g - b0)
                chunks.append((w, k, t))
                for j in range(k):
                    bi = done + b0 + j
                    blocks.append((t, bi == 0, bi == tile_blocks - 1))
                b0 += k
            done += g
    goff, boff = [], []
    g0 = b0_ = 0
    for (w, k, t) in chunks:
        goff.append(g0); boff.append(b0_)
        g0 += k * TILE // 16
        b0_ += k

    # per-core staged arrays
    cores = []
    for c in range(NCORES):
        s, d, t_id, w_id = per_core[c]
        gidx = np.zeros((b0_ * TILE,), np.int16)       # pad: window row 0
        dstl = np.full((b0_ * TILE,), -1.0, np.float32)  # pad: matches no dst
        # locate each core group inside the shared layout
        key = t_id * nwin + w_id
        cuts = np.flatnonzero(np.diff(key)) + 1
        starts = np.concatenate([[0], cuts]) if len(s) else np.array([], np.int64)
        ends = np.concatenate([cuts, [len(s)]]) if len(s) else np.array([], np.int64)
        # block offset of group (t, w) in the shared layout
        grp_boff = np.zeros((ntiles, nwin), np.int64)
        acc = 0
        for t in range(ntiles):
            for w in range(nwin):
                grp_boff[t, w] = acc
                acc += nb[t, w]
        for a, b in zip(starts, ends):
            t = int(t_id[a]); w = int(w_id[a])
            o = grp_boff[t, w] * TILE
            cnt = b - a
            gidx[o:o + cnt] = (s[a:b] - w * WIN).astype(np.int16)
            dstl[o:o + cnt] = (d[a:b] - t * TILE).astype(np.float32)
        cores.append(dict(
            gidx=wrap16x8(gidx),
            dstl=np.ascontiguousarray(
                dstl.reshape(b0_, TILE).T.astype(ml_dtypes.bfloat16)),
            base=c * shard,
        ))

    plan = dict(chunks=chunks, blocks=blocks, goff=goff, boff=boff,
                ntiles=ntiles, dpad=dpad, shard=shard, nwin=nwin,
                gcols=g0, bcols=b0_)
    return dinv, plan, cores


# ----------------------------------------------------------------------------
# device program
# ----------------------------------------------------------------------------

def emit_conv_edges(nc, pool, ipool, psum, plan, hbuf, gidx_d, dstl_d, iota_t,
                    dinvrep_t, bias_t, out_cb, out_dtype):
    """One conv's edge aggregation. out_cb(tile_idx, ap_or_tile)."""
    chunks = plan["chunks"]
    agg = {"t": None}
    sup = {}
    bi = 0
    for ci, (w, k, t) in enumerate(chunks):
        if ci % SG == 0:
            # coalesced index/dstl load for chunks [ci, ci+SG)
            hi = min(ci + SG, len(chunks))
            g0 = plan["goff"][ci]
            b0 = plan["boff"][ci]
            g1 = (plan["goff"][hi - 1] + chunks[hi - 1][1] * TILE // 16
                  if hi - 1 < len(chunks) else plan["gcols"])
            b1 = plan["boff"][hi - 1] + chunks[hi - 1][1]
            it = ipool.tile([128, SG * CB * TILE // 16], I16, tag="gidx")
            nc.sync.dma_start(it[:, :g1 - g0], gidx_d[:, g0:g1])
            dl = ipool.tile([128, SG * CB], BF16, tag="dstl")
            nc.sync.dma_start(dl[:, :b1 - b0], dstl_d[:, b0:b1])
            sup = {"it": it, "dl": dl, "g0": g0, "b0": b0}
        go = plan["goff"][ci] - sup["g0"]
        bo = plan["boff"][ci] - sup["b0"]
        it, dl = sup["it"], sup["dl"]
        nidx = k * TILE
        g = pool.tile([128, CB, 64], F32, tag="g")
        nc.gpsimd.dma_gather(
            g[:, :k, :],
            hbuf[w * WIN:(w + 1) * WIN, :],
            it[:, go:go + nidx // 16],
            num_idxs=nidx, num_idxs_reg=nidx, elem_size=64,
            queue_num=ci % NQ,
        )
        g2 = pool.tile([128, CB, 64], BF16, tag="g2")
        nc.scalar.activation(g2[:, :k, :], g[:, :k, :],
                             mybir.ActivationFunctionType.Copy)
        s_t = pool.tile([128, CB, TILE], BF16, tag="s")
        nc.vector.tensor_tensor(
            s_t[:, :k, :],
            iota_t[:].unsqueeze(1).broadcast_to([128, k, TILE]),
            dl[:, bo:bo + k].unsqueeze(2).broadcast_to([128, k, TILE]),
            op=mybir.AluOpType.is_equal,
        )
        for j in range(k):
            t_, start, stop = plan["blocks"][bi]; bi += 1
            if start:
                agg["t"] = psum.tile([64, TILE], F32, tag="agg", name=f"agg_{bi}")
            nc.tensor.matmul(agg["t"][:], lhsT=g2[:, j, :], rhs=s_t[:, j, :],
                             start=start, stop=stop)
            if stop:
                ag = agg["t"]
                e1 = pool.tile([64, TILE], F32, tag="ep1")
                nc.vector.tensor_tensor(
                    e1[:], ag[:],
                    dinvrep_t[:, t_ * TILE:(t_ + 1) * TILE],
                    op=mybir.AluOpType.mult)
                e2 = pool.tile([64, TILE], out_dtype, tag="ep2")
                nc.vector.tensor_tensor(
                    e2[:], e1[:], bias_t[:].broadcast_to([64, TILE]),
                    op=mybir.AluOpType.add)
                out_cb(t_, e2)


def build_program(meta, plan):
    n = meta["n"]
    npad = meta["npad"]
    dpad = plan["dpad"]
    shard = plan["shard"]
    ntiles = plan["ntiles"]
    gcols = max(plan["gcols"], 16)
    bcols = max(plan["bcols"], 1)

    nc = bacc.Bacc("TRN2", target_bir_lowering=False, debug=False,
                   num_devices=NCORES, num_swdge_queues=NQ)

    xt = nc.dram_tensor("xt", [128, npad], BF16, kind="ExternalInput")
    h1buf = nc.dram_tensor("h1buf", [npad, 64], F32, kind="ExternalInput")
    h2buf = nc.dram_tensor("h2buf", [npad, 64], F32, kind="ExternalInput")
    gidx_d = nc.dram_tensor("gidx", [128, gcols], I16, kind="ExternalInput")
    dstl_d = nc.dram_tensor("dstl", [128, bcols], BF16, kind="ExternalInput")
    w1_d = nc.dram_tensor("w1", [128, 64], BF16, kind="ExternalInput")
    w2_d = nc.dram_tensor("w2", [64, 64], BF16, kind="ExternalInput")
    lw1_d = nc.dram_tensor("lw1", [64, 64], F32, kind="ExternalInput")
    lw2_d = nc.dram_tensor("lw2", [64, 32], F32, kind="ExternalInput")
    lw3_d = nc.dram_tensor("lw3", [32, 1], F32, kind="ExternalInput")
    b1_d = nc.dram_tensor("b1", [64, 1], F32, kind="ExternalInput")
    b2_d = nc.dram_tensor("b2", [64, 1], F32, kind="ExternalInput")
    lb1_d = nc.dram_tensor("lb1", [64, 1], F32, kind="ExternalInput")
    lb2_d = nc.dram_tensor("lb2", [32, 1], F32, kind="ExternalInput")
    lb3_d = nc.dram_tensor("lb3", [1, 1], F32, kind="ExternalInput")
    iota_d = nc.dram_tensor("iota", [128, TILE], BF16, kind="ExternalInput")
    dinvrep_d = nc.dram_tensor("dinvrep", [64, dpad], F32, kind="ExternalInput")
    dinvc_d = nc.dram_tensor("dinvc", [128, NCORES * ntiles], F32,
                             kind="ExternalInput")
    out_d = nc.dram_tensor("out", [1, dpad], F32, kind="ExternalOutput")

    AC = 512  # phase A/C node-chunk

    with tile.TileContext(nc) as tc:
        with (
            tc.tile_pool(name="const", bufs=1) as cpool,
            tc.tile_pool(name="work", bufs=6) as pool,
            tc.tile_pool(name="head", bufs=3) as hpool,
            tc.tile_pool(name="idx", bufs=3) as ipool,
            tc.tile_pool(name="xtp", bufs=2) as xtpool,
            tc.tile_pool(name="psag", bufs=2, space="PSUM") as psag,
            tc.tile_pool(name="psmm", bufs=4, space="PSUM") as psmm,
            tc.tile_pool(name="dram", bufs=1, space="DRAM") as dram,
        ):
            def load_const(dram_t, shape, dtype, tag):
                t = cpool.tile(shape, dtype, tag=tag)
                nc.sync.dma_start(t[:], dram_t[:])
                return t

            w1_t = load_const(w1_d, [128, 64], BF16, "w1")
            w2_t = load_const(w2_d, [64, 64], BF16, "w2")
            lw1_t = load_const(lw1_d, [64, 64], F32, "lw1")
            lw2_t = load_const(lw2_d, [64, 32], F32, "lw2")
            lw3_t = load_const(lw3_d, [32, 1], F32, "lw3")
            b1_t = load_const(b1_d, [64, 1], F32, "b1")
            b2_t = load_const(b2_d, [64, 1], F32, "b2")
            lb1_t = load_const(lb1_d, [64, 1], F32, "lb1")
            lb2_t = load_const(lb2_d, [32, 1], F32, "lb2")
            lb3_t = load_const(lb3_d, [1, 1], F32, "lb3")
            iota_t = load_const(iota_d, [128, TILE], BF16, "iota")
            dinvrep_t = load_const(dinvrep_d, [64, dpad], F32, "dinvrep")
            dinvc_t = load_const(dinvc_d, [128, NCORES * ntiles], F32, "dinvc")

            out_engines = [nc.scalar, nc.gpsimd, nc.sync]

            # --- phase A ---
            for t in range(npad // AC):
                st = xtpool.tile([128, AC], BF16, tag="xt")
                nc.sync.dma_start(st[:], xt[:, t * AC:(t + 1) * AC])
                for j in range(AC // TILE):
                    ps = psmm.tile([TILE, 64], F32, tag="mm")
                    nc.tensor.matmul(
                        ps[:], lhsT=st[:, j * TILE:(j + 1) * TILE],
                        rhs=w1_t[:], start=True, stop=True)
                    sb = pool.tile([TILE, 64], F32, tag="arow")
                    nc.vector.tensor_copy(sb[:], ps[:])
                    r = t * AC + j * TILE
                    out_engines[j % 3].dma_start(h1buf[r:r + TILE, :], sb[:])

            # --- conv1 edges -> h1T bf16 bounce ---
            h1t_bounce = dram.tile([64, dpad], BF16)
            ag_out = dram.tile([NCORES * 64, dpad], BF16, addr_space="Shared")

            def conv1_out(t_, e2):
                nc.scalar.dma_start(h1t_bounce[:, t_ * TILE:(t_ + 1) * TILE],
                                    e2[:])

            emit_conv_edges(nc, pool, ipool, psag, plan, h1buf, gidx_d, dstl_d,
                            iota_t, dinvrep_t, b1_t, conv1_out, BF16)

            if dpad > shard:
                zt = pool.tile([64, dpad - shard], BF16, tag="zt")
                nc.vector.memset(zt[:], 0.0)
                nc.sync.dma_start(h1t_bounce[:, shard:], zt[:])

            nc.gpsimd.collective_compute(
                "AllGather", mybir.AluOpType.bypass,
                ins=[h1t_bounce[:].opt()],
                outs=[ag_out[:].opt()],
                replica_groups=[list(range(NCORES))],
            )

            # --- phase C: h2l rows for all nodes ---
            for c in range(NCORES):
                for o in range(0, shard, AC):
                    cw = min(AC, shard - o)
                    st = xtpool.tile([64, AC], BF16, tag="ct")
                    nc.sync.dma_start(
                        st[:, :cw], ag_out[c * 64:(c + 1) * 64, o:o + cw])
                    for j in range(0, cw, TILE):
                        cnt = min(TILE, cw - j)
                        n0 = c * shard + o + j
                        t = (o + j) // TILE
                        ps = psmm.tile([TILE, 64], F32, tag="mm")
                        nc.tensor.matmul(ps[:cnt, :], lhsT=st[:, j:j + cnt],
                                         rhs=w2_t[:], start=True, stop=True)
                        sb = pool.tile([TILE, 64], F32, tag="crow")
                        nc.vector.tensor_tensor(
                            sb[:cnt, :], ps[:cnt, :],
                            dinvc_t[:cnt, c * ntiles + t:c * ntiles + t + 1]
                            .broadcast_to([cnt, 64]),
                            op=mybir.AluOpType.mult)
                        out_engines[(j // TILE) % 3].dma_start(
                            h2buf[n0:n0 + cnt, :], sb[:cnt, :])

            # --- conv2 edges -> h2T f32 in SBUF ---
            h2t_sb = cpool.tile([64, dpad], F32, tag="h2t")

            def conv2_out(t_, e2):
                nc.vector.tensor_copy(h2t_sb[:, t_ * TILE:(t_ + 1) * TILE],
                                      e2[:])

            emit_conv_edges(nc, pool, ipool, psag, plan, h2buf, gidx_d, dstl_d,
                            iota_t, dinvrep_t, b2_t, conv2_out, F32)

            # --- MLP head (transposed space) ---
            EC = 512
            for o in range(0, dpad, EC):
                w_ = min(EC, dpad - o)
                p1 = psmm.tile([64, EC], F32, tag="mm")
                nc.tensor.matmul(p1[:, :w_], lhsT=lw1_t[:],
                                 rhs=h2t_sb[:, o:o + w_], start=True, stop=True)
                z1 = hpool.tile([64, EC], F32, tag="z1")
                nc.scalar.activation(z1[:, :w_], p1[:, :w_],
                                     mybir.ActivationFunctionType.Relu,
                                     bias=lb1_t[:])
                p2 = psmm.tile([32, EC], F32, tag="mm")
                nc.tensor.matmul(p2[:, :w_], lhsT=lw2_t[:], rhs=z1[:, :w_],
                                 start=True, stop=True)
                z2 = hpool.tile([32, EC], F32, tag="z2")
                nc.scalar.activation(z2[:, :w_], p2[:, :w_],
                                     mybir.ActivationFunctionType.Relu,
                                     bias=lb2_t[:])
                p3 = psmm.tile([1, EC], F32, tag="mm")
                nc.tensor.matmul(p3[:, :w_], lhsT=lw3_t[:], rhs=z2[:, :w_],
                                 start=True, stop=True)
                z3 = hpool.tile([1, EC], F32, tag="z3")
                nc.vector.tensor_tensor(z3[:, :w_], p3[:, :w_],
                                        lb3_t[:].broadcast_to([1, w_]),
                                        op=mybir.AluOpType.add)
                nc.sync.dma_start(out_d[:, o:o + w_], z3[:, :w_])

    nc.compile()
    return nc


# ----------------------------------------------------------------------------
# entry point
# ----------------------------------------------------------------------------

def kernel(x, edge_index, W1, b1, W2, b2, lw1, lb1, lw2, lb2, lw3, lb3,
           _want_trace=False):
    x = np.asarray(x, np.float32)
    edge_index = np.asarray(edge_index)
    n = x.shape[0]
    npad = ((n + WIN - 1) // WIN) * WIN

    dinv, plan, cores = preprocess(n, edge_index)
    shard, dpad, ntiles = plan["shard"], plan["dpad"], plan["ntiles"]

    xt = np.zeros((128, npad), ml_dtypes.bfloat16)
    xt[:, :n] = (x * dinv[:, None]).T.astype(ml_dtypes.bfloat16)
    hz = np.zeros((npad, 64), np.float32)
    iota = np.tile(np.arange(TILE, dtype=np.float32), (128, 1)).astype(
        ml_dtypes.bfloat16)

    dinvc = np.zeros((128, NCORES * ntiles), np.float32)
    for cc in range(NCORES):
        for t in range(ntiles):
            n0 = cc * shard + t * TILE
            cnt = min(TILE, (cc + 1) * shard - n0)
            dinvc[:cnt, cc * ntiles + t] = dinv[n0:n0 + cnt]

    in_maps = []
    for c in range(NCORES):
        dinvrep = np.zeros((64, dpad), np.float32)
        dinvrep[:, :shard] = dinv[c * shard:(c + 1) * shard][None, :]
        in_maps.append({
            "xt": xt, "h1buf": hz, "h2buf": hz,
            "gidx": cores[c]["gidx"], "dstl": cores[c]["dstl"],
            "w1": np.asarray(W1, np.float32).astype(ml_dtypes.bfloat16),
            "w2": np.asarray(W2, np.float32).astype(ml_dtypes.bfloat16),
            "lw1": np.ascontiguousarray(np.asarray(lw1, np.float32)),
            "lw2": np.ascontiguousarray(np.asarray(lw2, np.float32)),
            "lw3": np.ascontiguousarray(np.asarray(lw3, np.float32)),
            "b1": np.asarray(b1, np.float32).reshape(-1, 1),
            "b2": np.asarray(b2, np.float32).reshape(-1, 1),
            "lb1": np.asarray(lb1, np.float32).reshape(-1, 1),
            "lb2": np.asarray(lb2, np.float32).reshape(-1, 1),
            "lb3": np.asarray(lb3, np.float32).reshape(-1, 1),
            "iota": iota, "dinvrep": dinvrep, "dinvc": dinvc,
        })

    meta = {"n": n, "npad": npad}
    nc = build_program(meta, plan)

    res = run_bass_kernel_spmd(nc, in_maps, core_ids=list(range(NCORES)),
                               trace=_want_trace)
    out = np.empty((n, 1), np.float32)
    for c in range(NCORES):
        out[c * shard:(c + 1) * shard, 0] = res.results[c]["out"][0, :shard]
    kernel._last_exec_ns = res.exec_time_ns
    return out


# revision 10
# speedup vs baseline: 3.0414x; 1.4574x over previous
"""GCN (2x GCNConv + MLP head) on 8 TRN2 NeuronCores via Bass/Tile.

Distribution (graph-parallel, per the node-sharding scheme):
  - nodes sharded by id across 8 cores (12500 each); weights replicated.
  - Phase A (replicated): h1l rows = (dinv*x) @ W1 for ALL nodes -> DRAM.
  - Conv edge phase (sharded by dst): for each core's in-edges,
    dma_gather 256B message rows by src id (4 SWDGE queues round-robin;
    pad slots carry idx=-1 so they emit no DMA descriptor), scalar-engine
    copy to bf16, then per-128-edge block a DVE-built bf16 one-hot S_dst
    ([128, 64] dst tiles) and a bf16 PE matmul accumulate aggT[64, 64]
    per dst tile in PSUM; epilogue dinv*aggT on DVE + bias on ACT.
  - AllGather of h1T shards (bf16) = the halo exchange.
  - Phase C (replicated): h2l rows = h1 @ W2 for ALL nodes -> DRAM.
  - Conv2 edge phase -> h2T (f32, SBUF resident).
  - MLP head in transposed space; output row [1, shard].

Host preprocessing is structure-only (derived from edge_index): degrees,
edge blocking by (dst-tile, src-window), int16 gather indices. All cores
share one program: block structure is padded to the max across cores.
Pad slots have gidx=-1 (no descriptor) and dstl=-1 (one-hot zeroes them).
"""

import numpy as np
import ml_dtypes

import concourse.bass as bass
import concourse.bacc as bacc
import concourse.tile as tile
import concourse.mybir as mybir
from concourse.bass_utils import run_bass_kernel_spmd

F32 = mybir.dt.float32
BF16 = mybir.dt.bfloat16
I16 = mybir.dt.int16

NCORES = 8
WIN = 25088          # gather window rows (multiple of 128, < int16 max)
EB = 128             # edges per block (PE contraction height)
DTILE = 64           # dst tile width (one-hot cols, PSUM agg cols)
CB = 8               # max blocks per dma_gather (1024-idx HW limit)
NQ = 4               # SWDGE queues (ucode max)
SG = 8               # chunks per coalesced index-load supergroup


# ----------------------------------------------------------------------------
# host-side preprocessing (numpy only)
# ----------------------------------------------------------------------------

def wrap16x8(a):
    """[n] int16 -> [128, n//16]: idx i at [i%16, i//16], replicated x8."""
    w = np.ascontiguousarray(np.transpose(a.reshape(-1, 16), (1, 0)))
    return np.ascontiguousarray(np.tile(w, (8, 1)))


def preprocess(n, edge_index):
    """Uniform cross-core edge plan.

    Returns (dinv, plan, cores) where plan holds the shared structure
    (chunks/blocks/flags) and cores[c] holds per-core staged index arrays.
    """
    src = edge_index[0].astype(np.int64)
    dst = edge_index[1].astype(np.int64)

    deg = np.bincount(dst, minlength=n).astype(np.float64) + 1.0
    dinv = (1.0 / np.sqrt(deg)).astype(np.float32)

    shard = n // NCORES
    assert shard * NCORES == n and shard % 2 == 0
    ntiles = (shard + DTILE - 1) // DTILE
    dpad = ntiles * DTILE
    nwin = (n + WIN - 1) // WIN

    loops = np.arange(n, dtype=np.int64)
    src = np.concatenate([src, loops])
    dst = np.concatenate([dst, loops])

    # per-core edge lists grouped by (dst tile, src window)
    per_core = []
    counts = np.zeros((NCORES, ntiles, nwin), np.int64)
    for c in range(NCORES):
        base = c * shard
        m = (dst >= base) & (dst < base + shard)
        s, d = src[m], dst[m] - base
        t_id = d // DTILE
        w_id = s // WIN
        order = np.lexsort((w_id, t_id))
        s, d, t_id, w_id = s[order], d[order], t_id[order], w_id[order]
        np.add.at(counts[c], (t_id, w_id), 1)
        per_core.append((s, d, t_id, w_id))

    nb = (counts.max(axis=0) + EB - 1) // EB      # [ntiles, nwin] blocks

    # shared chunk/block structure, tile-major
    chunks = []   # (window, n_blocks, tile)
    blocks = []   # (tile, start, stop)
    for t in range(ntiles):
        tile_blocks = int(nb[t].sum())
        done = 0
        for w in range(nwin):
            g = int(nb[t, w])
            b0 = 0
            while b0 < g:
                k = min(CB, g - b0)
                chunks.append((w, k, t))
                for j in range(k):
                    bi = done + b0 + j
                    blocks.append((t, bi == 0, bi == tile_blocks - 1))
                b0 += k
            done += g
    goff, boff, nidxs = [], [], []
    g0 = b0_ = 0
    cmax = counts.max(axis=0)                      # [ntiles, nwin]
    done_in_grp = {}
    for (w, k, t) in chunks:
        goff.append(g0); boff.append(b0_)
        b0 = done_in_grp.get((t, w), 0)
        real = int(min(max(cmax[t, w] - b0 * EB, 1), k * EB))
        nidxs.append((real + 15) // 16 * 16)
        done_in_grp[(t, w)] = b0 + k
        g0 += k * EB // 16
        b0_ += k

    # per-core staged arrays
    cores = []
    for c in range(NCORES):
        s, d, t_id, w_id = per_core[c]
        gidx = np.zeros((b0_ * EB,), np.int16)           # pad: window row 0
        dstl = np.full((b0_ * EB,), -1.0, np.float32)    # pad: matches no dst
        # locate each core group inside the shared layout
        key = t_id * nwin + w_id
        cuts = np.flatnonzero(np.diff(key)) + 1
        starts = np.concatenate([[0], cuts]) if len(s) else np.array([], np.int64)
        ends = np.concatenate([cuts, [len(s)]]) if len(s) else np.array([], np.int64)
        # block offset of group (t, w) in the shared layout
        grp_boff = np.zeros((ntiles, nwin), np.int64)
        acc = 0
        for t in range(ntiles):
            for w in range(nwin):
                grp_boff[t, w] = acc
                acc += nb[t, w]
        for a, b in zip(starts, ends):
            t = int(t_id[a]); w = int(w_id[a])
            o = grp_boff[t, w] * EB
            cnt = b - a
            gidx[o:o + cnt] = (s[a:b] - w * WIN).astype(np.int16)
            dstl[o:o + cnt] = (d[a:b] - t * DTILE).astype(np.float32)
        cores.append(dict(
            gidx=wrap16x8(gidx),
            dstl=np.ascontiguousarray(
                dstl.reshape(b0_, EB).T.astype(ml_dtypes.bfloat16)),
            base=c * shard,
        ))

    plan = dict(chunks=chunks, blocks=blocks, goff=goff, boff=boff,
                nidxs=nidxs, ntiles=ntiles, dpad=dpad, shard=shard,
                nwin=nwin, gcols=g0, bcols=b0_)
    return dinv, plan, cores


# ----------------------------------------------------------------------------
# device program
# ----------------------------------------------------------------------------

def emit_conv_edges(nc, pool, ipool, psum, plan, hbuf, gidx_d, dstl_d, iota_t,
                    dinvrep_t, bias_t, out_cb):
    """One conv's edge aggregation. out_cb(tile_idx, e1_f32_tile)."""
    chunks = plan["chunks"]
    agg = {"t": None}
    sup = {}
    bi = 0
    for ci, (w, k, t) in enumerate(chunks):
        if ci % SG == 0:
            # coalesced index/dstl load for chunks [ci, ci+SG)
            hi = min(ci + SG, len(chunks))
            g0 = plan["goff"][ci]
            b0 = plan["boff"][ci]
            g1 = plan["goff"][hi - 1] + chunks[hi - 1][1] * EB // 16
            b1 = plan["boff"][hi - 1] + chunks[hi - 1][1]
            it = ipool.tile([128, SG * CB * EB // 16], I16, tag="gidx")
            nc.sync.dma_start(it[:, :g1 - g0], gidx_d[:, g0:g1])
            dl = ipool.tile([128, SG * CB], BF16, tag="dstl")
            nc.sync.dma_start(dl[:, :b1 - b0], dstl_d[:, b0:b1])
            sup = {"it": it, "dl": dl, "g0": g0, "b0": b0}
        go = plan["goff"][ci] - sup["g0"]
        bo = plan["boff"][ci] - sup["b0"]
        it, dl = sup["it"], sup["dl"]
        nidx = plan["nidxs"][ci]
        kk = (nidx + EB - 1) // EB
        g = pool.tile([128, CB, 64], F32, tag="g")
        nc.gpsimd.dma_gather(
            g[:, :kk, :],
            hbuf[w * WIN:(w + 1) * WIN, :],
            it[:, go:go + nidx // 16],
            num_idxs=nidx, num_idxs_reg=nidx, elem_size=64,
            queue_num=ci % NQ,
        )
        g2 = pool.tile([128, CB, 64], BF16, tag="g2")
        nc.scalar.activation(g2[:, :k, :], g[:, :k, :],
                             mybir.ActivationFunctionType.Copy)
        s_t = pool.tile([128, CB, DTILE], BF16, tag="s")
        nc.vector.tensor_tensor(
            s_t[:, :k, :],
            iota_t[:].unsqueeze(1).broadcast_to([128, k, DTILE]),
            dl[:, bo:bo + k].unsqueeze(2).broadcast_to([128, k, DTILE]),
            op=mybir.AluOpType.is_equal,
        )
        for j in range(k):
            t_, start, stop = plan["blocks"][bi]; bi += 1
            if start:
                agg["t"] = psum.tile([64, DTILE], F32, tag="agg",
                                     name=f"agg_{bi}")
            nc.tensor.matmul(agg["t"][:], lhsT=g2[:, j, :], rhs=s_t[:, j, :],
                             start=start, stop=stop)
            if stop:
                ag = agg["t"]
                e1 = pool.tile([64, DTILE], F32, tag="ep1")
                nc.vector.tensor_tensor(
                    e1[:], ag[:],
                    dinvrep_t[:, t_ * DTILE:(t_ + 1) * DTILE],
                    op=mybir.AluOpType.mult)
                out_cb(t_, e1)


def build_program(meta, plan):
    n = meta["n"]
    npad = meta["npad"]
    dpad = plan["dpad"]
    shard = plan["shard"]
    ntiles = plan["ntiles"]
    nptiles = dpad // 128
    gcols = max(plan["gcols"], 16)
    bcols = max(plan["bcols"], 1)

    nc = bacc.Bacc("TRN2", target_bir_lowering=False, debug=False,
                   num_devices=NCORES, num_swdge_queues=NQ)

    xt = nc.dram_tensor("xt", [128, npad], BF16, kind="ExternalInput")
    h1buf = nc.dram_tensor("h1buf", [npad, 64], F32, kind="ExternalInput")
    h2buf = nc.dram_tensor("h2buf", [npad, 64], F32, kind="ExternalInput")
    gidx_d = nc.dram_tensor("gidx", [128, gcols], I16, kind="ExternalInput")
    dstl_d = nc.dram_tensor("dstl", [128, bcols], BF16, kind="ExternalInput")
    w1_d = nc.dram_tensor("w1", [128, 64], BF16, kind="ExternalInput")
    w2_d = nc.dram_tensor("w2", [64, 64], BF16, kind="ExternalInput")
    lw1_d = nc.dram_tensor("lw1", [64, 64], F32, kind="ExternalInput")
    lw2_d = nc.dram_tensor("lw2", [64, 32], F32, kind="ExternalInput")
    lw3_d = nc.dram_tensor("lw3", [32, 1], F32, kind="ExternalInput")
    b1_d = nc.dram_tensor("b1", [64, 1], F32, kind="ExternalInput")
    b2_d = nc.dram_tensor("b2", [64, 1], F32, kind="ExternalInput")
    lb1_d = nc.dram_tensor("lb1", [64, 1], F32, kind="ExternalInput")
    lb2_d = nc.dram_tensor("lb2", [32, 1], F32, kind="ExternalInput")
    lb3_d = nc.dram_tensor("lb3", [1, 1], F32, kind="ExternalInput")
    iota_d = nc.dram_tensor("iota", [128, DTILE], BF16, kind="ExternalInput")
    dinvrep_d = nc.dram_tensor("dinvrep", [64, dpad], F32, kind="ExternalInput")
    dinvc_d = nc.dram_tensor("dinvc", [128, NCORES * nptiles], F32,
                             kind="ExternalInput")
    out_d = nc.dram_tensor("out", [1, dpad], F32, kind="ExternalOutput")

    AC = 512  # phase A/C node-chunk

    with tile.TileContext(nc) as tc:
        with (
            tc.tile_pool(name="const", bufs=1) as cpool,
            tc.tile_pool(name="work", bufs=6) as pool,
            tc.tile_pool(name="head", bufs=3) as hpool,
            tc.tile_pool(name="idx", bufs=3) as ipool,
            tc.tile_pool(name="xtp", bufs=2) as xtpool,
            tc.tile_pool(name="psag", bufs=2, space="PSUM") as psag,
            tc.tile_pool(name="psmm", bufs=2, space="PSUM") as psmm,
            tc.tile_pool(name="pshd", bufs=1, space="PSUM") as pshd,
            tc.tile_pool(name="dram", bufs=1, space="DRAM") as dram,
        ):
            def load_const(dram_t, shape, dtype, tag):
                t = cpool.tile(shape, dtype, tag=tag)
                nc.sync.dma_start(t[:], dram_t[:])
                return t

            w1_t = load_const(w1_d, [128, 64], BF16, "w1")
            w2_t = load_const(w2_d, [64, 64], BF16, "w2")
            lw1_t = load_const(lw1_d, [64, 64], F32, "lw1")
            lw2_t = load_const(lw2_d, [64, 32], F32, "lw2")
            lw3_t = load_const(lw3_d, [32, 1], F32, "lw3")
            b1_t = load_const(b1_d, [64, 1], F32, "b1")
            b2_t = load_const(b2_d, [64, 1], F32, "b2")
            lb1_t = load_const(lb1_d, [64, 1], F32, "lb1")
            lb2_t = load_const(lb2_d, [32, 1], F32, "lb2")
            lb3_t = load_const(lb3_d, [1, 1], F32, "lb3")
            iota_t = load_const(iota_d, [128, DTILE], BF16, "iota")
            dinvrep_t = load_const(dinvrep_d, [64, dpad], F32, "dinvrep")
            dinvc_t = load_const(dinvc_d, [128, NCORES * nptiles], F32, "dinvc")

            out_engines = [nc.scalar, nc.gpsimd, nc.sync]

            # --- phase A ---
            for t in range(npad // AC):
                st = xtpool.tile([128, AC], BF16, tag="xt")
                nc.sync.dma_start(st[:], xt[:, t * AC:(t + 1) * AC])
                for j in range(AC // 128):
                    ps = psmm.tile([128, 64], F32, tag="mm")
                    nc.tensor.matmul(
                        ps[:], lhsT=st[:, j * 128:(j + 1) * 128],
                        rhs=w1_t[:], start=True, stop=True)
                    sb = pool.tile([128, 64], F32, tag="arow")
                    nc.scalar.activation(sb[:], ps[:],
                                         mybir.ActivationFunctionType.Copy)
                    r = t * AC + j * 128
                    out_engines[j % 3].dma_start(h1buf[r:r + 128, :], sb[:])

            # --- conv1 edges -> h1T bf16 bounce ---
            h1t_bounce = dram.tile([64, dpad], BF16)
            ag_out = dram.tile([NCORES * 64, dpad], BF16, addr_space="Shared")

            def conv1_out(t_, e1):
                e2 = pool.tile([64, DTILE], BF16, tag="ep2")
                nc.scalar.activation(e2[:], e1[:],
                                     mybir.ActivationFunctionType.Identity,
                                     bias=b1_t[:])
                nc.scalar.dma_start(h1t_bounce[:, t_ * DTILE:(t_ + 1) * DTILE],
                                    e2[:])

            emit_conv_edges(nc, pool, ipool, psag, plan, h1buf, gidx_d, dstl_d,
                            iota_t, dinvrep_t, b1_t, conv1_out)

            if dpad > shard:
                zt = pool.tile([64, dpad - shard], BF16, tag="zt")
                nc.vector.memset(zt[:], 0.0)
                nc.sync.dma_start(h1t_bounce[:, shard:], zt[:])

            nc.gpsimd.collective_compute(
                "AllGather", mybir.AluOpType.bypass,
                ins=[h1t_bounce[:].opt()],
                outs=[ag_out[:].opt()],
                replica_groups=[list(range(NCORES))],
            )

            # --- phase C: h2l rows for all nodes ---
            for c in range(NCORES):
                for o in range(0, shard, AC):
                    cw = min(AC, shard - o)
                    st = xtpool.tile([64, AC], BF16, tag="ct")
                    nc.sync.dma_start(
                        st[:, :cw], ag_out[c * 64:(c + 1) * 64, o:o + cw])
                    for j in range(0, cw, 128):
                        cnt = min(128, cw - j)
                        n0 = c * shard + o + j
                        t = (o + j) // 128
                        ps = psmm.tile([128, 64], F32, tag="mm")
                        nc.tensor.matmul(ps[:cnt, :], lhsT=st[:, j:j + cnt],
                                         rhs=w2_t[:], start=True, stop=True)
                        sb = pool.tile([128, 64], F32, tag="crow")
                        nc.vector.tensor_tensor(
                            sb[:cnt, :], ps[:cnt, :],
                            dinvc_t[:cnt, c * nptiles + t:c * nptiles + t + 1]
                            .broadcast_to([cnt, 64]),
                            op=mybir.AluOpType.mult)
                        out_engines[(j // 128) % 3].dma_start(
                            h2buf[n0:n0 + cnt, :], sb[:cnt, :])

            # --- conv2 edges -> h2T f32 in SBUF ---
            h2t_sb = cpool.tile([64, dpad], F32, tag="h2t")

            def conv2_out(t_, e1):
                nc.scalar.activation(h2t_sb[:, t_ * DTILE:(t_ + 1) * DTILE],
                                     e1[:],
                                     mybir.ActivationFunctionType.Identity,
                                     bias=b2_t[:])

            emit_conv_edges(nc, pool, ipool, psag, plan, h2buf, gidx_d, dstl_d,
                            iota_t, dinvrep_t, b2_t, conv2_out)

            # --- MLP head (transposed space) ---
            EC = 512
            for o in range(0, dpad, EC):
                w_ = min(EC, dpad - o)
                p1 = pshd.tile([64, EC], F32, tag="mm1")
                nc.tensor.matmul(p1[:, :w_], lhsT=lw1_t[:],
                                 rhs=h2t_sb[:, o:o + w_], start=True, stop=True)
                z1 = hpool.tile([64, EC], F32, tag="z1")
                nc.scalar.activation(z1[:, :w_], p1[:, :w_],
                                     mybir.ActivationFunctionType.Relu,
                                     bias=lb1_t[:])
                p2 = pshd.tile([32, EC], F32, tag="mm2")
                nc.tensor.matmul(p2[:, :w_], lhsT=lw2_t[:], rhs=z1[:, :w_],
                                 start=True, stop=True)
                z2 = hpool.tile([32, EC], F32, tag="z2")
                nc.scalar.activation(z2[:, :w_], p2[:, :w_],
                                     mybir.ActivationFunctionType.Relu,
                                     bias=lb2_t[:])
                p3 = pshd.tile([1, EC], F32, tag="mm3")
                nc.tensor.matmul(p3[:, :w_], lhsT=lw3_t[:], rhs=z2[:, :w_],
                                 start=True, stop=True)
                z3 = hpool.tile([1, EC], F32, tag="z3")
                nc.vector.tensor_tensor(z3[:, :w_], p3[:, :w_],
                                        lb3_t[:].broadcast_to([1, w_]),
                                        op=mybir.AluOpType.add)
                nc.sync.dma_start(out_d[:, o:o + w_], z3[:, :w_])

    nc.compile()
    return nc


# ----------------------------------------------------------------------------
# entry point
# ----------------------------------------------------------------------------

def kernel(x, edge_index, W1, b1, W2, b2, lw1, lb1, lw2, lb2, lw3, lb3,
           _want_trace=False):
    x = np.asarray(x, np.float32)
    edge_index = np.asarray(edge_index)
    n = x.shape[0]
    npad = ((n + WIN - 1) // WIN) * WIN

    dinv, plan, cores = preprocess(n, edge_index)
    shard, dpad, ntiles = plan["shard"], plan["dpad"], plan["ntiles"]
    nptiles = dpad // 128

    xt = np.zeros((128, npad), ml_dtypes.bfloat16)
    xt[:, :n] = (x * dinv[:, None]).T.astype(ml_dtypes.bfloat16)
    hz = np.zeros((npad, 64), np.float32)
    iota = np.tile(np.arange(DTILE, dtype=np.float32), (128, 1)).astype(
        ml_dtypes.bfloat16)

    dinvc = np.zeros((128, NCORES * nptiles), np.float32)
    for cc in range(NCORES):
        for t in range(nptiles):
            n0 = cc * shard + t * 128
            cnt = min(128, (cc + 1) * shard - n0)
            dinvc[:cnt, cc * nptiles + t] = dinv[n0:n0 + cnt]

    in_maps = []
    for c in range(NCORES):
        dinvrep = np.zeros((64, dpad), np.float32)
        dinvrep[:, :shard] = dinv[c * shard:(c + 1) * shard][None, :]
        in_maps.append({
            "xt": xt, "h1buf": hz, "h2buf": hz,
            "gidx": cores[c]["gidx"], "dstl": cores[c]["dstl"],
            "w1": np.asarray(W1, np.float32).astype(ml_dtypes.bfloat16),
            "w2": np.asarray(W2, np.float32).astype(ml_dtypes.bfloat16),
            "lw1": np.ascontiguousarray(np.asarray(lw1, np.float32)),
            "lw2": np.ascontiguousarray(np.asarray(lw2, np.float32)),
            "lw3": np.ascontiguousarray(np.asarray(lw3, np.float32)),
            "b1": np.asarray(b1, np.float32).reshape(-1, 1),
            "b2": np.asarray(b2, np.float32).reshape(-1, 1),
            "lb1": np.asarray(lb1, np.float32).reshape(-1, 1),
            "lb2": np.asarray(lb2, np.float32).reshape(-1, 1),
            "lb3": np.asarray(lb3, np.float32).reshape(-1, 1),
            "iota": iota, "dinvrep": dinvrep, "dinvc": dinvc,
        })

    meta = {"n": n, "npad": npad}
    nc = build_program(meta, plan)

    res = run_bass_kernel_spmd(nc, in_maps, core_ids=list(range(NCORES)),
                               trace=_want_trace)
    out = np.empty((n, 1), np.float32)
    for c in range(NCORES):
        out[c * shard:(c + 1) * shard, 0] = res.results[c]["out"][0, :shard]
    kernel._last_exec_ns = res.exec_time_ns
    return out
